# revision 30
# baseline (speedup 1.0000x reference)
import numpy as np
import concourse.bass as bass
import concourse.mybir as mybir
from concourse.bass_utils import run_bass_kernel_spmd
from concourse.tile import TileContext
from concourse.mybir import AluOpType as Alu, ActivationFunctionType as Act

B, T, D, H, hd, SC, ST = 2, 2048, 1024, 16, 64, 64, 16
BT = B * T          # 4096
NC = 8
TOK = BT // NC      # 512 tokens per core
EPS = 1.1920929e-07
F32 = mybir.dt.float32
BF16 = mybir.dt.bfloat16


def _split_multi_waits(nc, max_waits=1):
    # this walrus build accepts only one sync wait per ISA instruction
    n = 0
    for f in nc.m.functions:
        for bb in f.blocks:
            out = []
            for inst in bb.instructions:
                si = inst.sync_info
                if si is not None and si.on_wait and len(si.on_wait) > max_waits:
                    for w in si.on_wait[:-max_waits]:
                        out.append(mybir.InstNoOp(
                            name=f"{inst.name}_ws{n}", ins=[], outs=[],
                            engine=inst.engine,
                            sync_info=mybir.SyncInfo(on_wait=[w], on_update=[]),
                            bass_nofuse=True))
                        n += 1
                    inst.sync_info = mybir.SyncInfo(
                        on_wait=si.on_wait[-max_waits:], on_update=si.on_update)
                out.append(inst)
            bb.instructions = out
    return n


def _build():
    nc = bass.Bass()
    dt = mybir.dt.float32r if int(__import__("os").environ.get("BASS_F32R", "1")) else F32

    # pre-tiled inputs: [128, k*W+j] layouts so each stream is one DMA
    xTt = nc.dram_tensor("xTt", [128, 8, BT], BF16, kind="ExternalInput")
    xmyT2 = nc.dram_tensor("xmyT2", [128, 8 * TOK], dt, kind="ExternalInput")
    qkvWt = nc.dram_tensor("qkvWt", [128, 8 * 384], BF16, kind="ExternalInput")
    oWt = nc.dram_tensor("oWt", [128, 8 * D], BF16, kind="ExternalInput")
    inWt = nc.dram_tensor("inWt", [128, 8 * SC], BF16, kind="ExternalInput")
    gateWt = nc.dram_tensor("gateWt", [128, 8 * SC], BF16, kind="ExternalInput")
    out_wT = nc.dram_tensor("out_wT", [SC, D], BF16, kind="ExternalInput")
    dt_wT = nc.dram_tensor("dt_wT", [SC, SC], BF16, kind="ExternalInput")
    BpT = nc.dram_tensor("BpT", [SC, ST], BF16, kind="ExternalInput")
    CpT = nc.dram_tensor("CpT", [SC, ST], BF16, kind="ExternalInput")
    dtb = nc.dram_tensor("dtb", [SC, 1], F32, kind="ExternalInput")
    alog = nc.dram_tensor("alog", [128, 8], F32, kind="ExternalInput")
    cpkB = nc.dram_tensor("cpkB", [128, 384], BF16, kind="ExternalInput")
    ej = nc.dram_tensor("ej", [8, 1024], dt, kind="ExternalInput")
    esc = nc.dram_tensor("esc", [SC, 1024], BF16, kind="ExternalInput")
    est = nc.dram_tensor("est", [ST, 128], BF16, kind="ExternalInput")
    r8 = nc.dram_tensor("r8", [128, 512], BF16, kind="ExternalInput")
    csel = nc.dram_tensor("csel", [128, 8], F32, kind="ExternalInput")
    omc = nc.dram_tensor("omc", [128, 64], F32, kind="ExternalInput")
    epsb = nc.dram_tensor("epsb", [128, 1], F32, kind="ExternalInput")

    yout = nc.dram_tensor("yout", [D, TOK], dt, kind="ExternalOutput")

    with nc.allow_low_precision(reason="bf16 matmul paths; f32r is fp32 bytes"), \
         TileContext(nc) as tc:
        with tc.tile_pool(name="const", bufs=1) as cpool, \
             tc.tile_pool(name="wts", bufs=1) as wpool, \
             tc.tile_pool(name="ow", bufs=1) as owp, \
             tc.tile_pool(name="xmy", bufs=1) as xmp, \
             tc.tile_pool(name="work", bufs=2) as work, \
             tc.tile_pool(name="psA", bufs=2, space="PSUM") as psA, \
             tc.tile_pool(name="psB", bufs=3, space="PSUM") as psB, \
             tc.tile_pool(name="psC", bufs=2, space="PSUM") as psC, \
             tc.tile_pool(name="dram", bufs=1, space="DRAM") as dram:

            # ---- highest-priority DMAs first (single triggers each) ----
            xtmyA = xmp.tile([128, 8 * TOK], dt, name="xtmyA", tag="xtmyA")
            nc.sync.dma_start(xtmyA[:, :], xmyT2[:, :])

            # packed bf16 consts: ident | tri | ones
            cpkS = cpool.tile([128, 384], BF16, name="cpkS", tag="cpkS")
            nc.sync.dma_start(cpkS[:, :], cpkB[:, :])
            identS = cpkS[:, 0:128]
            triS = cpkS[:, 128:256]
            onesS = cpkS[:, 256:384]

            def csbuf(shape, src, name, d=None):
                t = cpool.tile(shape, d or dt, name=name, tag=name)
                nc.sync.dma_start(t[:, :], src)
                return t

            ejS = csbuf([8, 1024], ej[:, :], "ejS")
            escS = csbuf([SC, 1024], esc[:, :], "escS", BF16)
            estS = csbuf([ST, 128], est[:, :], "estS", BF16)
            r8S = csbuf([128, 512], r8[:, :], "r8S", BF16)
            cselS = csbuf([128, 8], csel[:, :], "cselS", F32)
            omcS = csbuf([128, 64], omc[:, :], "omcS", F32)
            dtbS = csbuf([SC, 1], dtb[:, :], "dtbS", F32)
            alogS = csbuf([128, 8], alog[:, :], "alogS", F32)
            epsS = csbuf([128, 1], epsb[:, :], "epsS", F32)

            zerosF = cpool.tile([128, 512], F32, name="zerosF", tag="zerosF")
            nc.vector.memset(zerosF[:, :], 0.0)
            negA = cpool.tile([128, 8], F32, name="negA", tag="negA")
            nc.scalar.activation(negA[:, :], alogS[:, :], Act.Exp)
            nc.vector.tensor_scalar_mul(negA[:, :], negA[:, :], -1.0)

            # persistent post-phase weight tiles (DMAs issued later)
            inW = wpool.tile([128, 8 * SC], BF16, name="inW", tag="inW")
            gateW = wpool.tile([128, 8 * SC], BF16, name="gateW", tag="gateW")
            outW = wpool.tile([SC, D], BF16, name="outW", tag="outW")
            dtW = wpool.tile([SC, SC], BF16, name="dtW", tag="dtW")
            BpS = wpool.tile([SC, ST], BF16, name="BpS", tag="BpS")
            CpS = wpool.tile([SC, ST], BF16, name="CpS", tag="CpS")
            # o_proj weights on the gpsimd queue (one trigger)
            oW = owp.tile([128, 8 * D], BF16, name="oW", tag="oW")
            nc.gpsimd.dma_start(oW[:, :], oWt[:, :])

            # dram collective buffers
            cin1 = dram.tile([1, 512], BF16, name="cin1T", tag="cin1T")
            cout1 = dram.tile([NC, 512], BF16, name="cout1T", tag="cout1T")
            cin = dram.tile([NC, 128, TOK], BF16, name="cinT", tag="cinT")
            cout = dram.tile([NC, 128, TOK], BF16, name="coutT", tag="coutT")
            cin2 = dram.tile([128, 16], F32, name="cin2T", tag="cin2T")
            cout2 = dram.tile([NC * 128, 16], F32, name="cout2T", tag="cout2T")

            with tc.tile_pool(name="attn", bufs=1) as apool:
                qkvW = apool.tile([128, 8 * 384], BF16, name="qkvW", tag="qkvW")
                nc.sync.dma_start(qkvW[:, :], qkvWt[:, :])
                Qf = [apool.tile([128, T], BF16, name=f"Qf{b}", tag=f"Qf{b}")
                      for b in range(B)]
                Kf = [apool.tile([128, T], BF16, name=f"Kf{b}", tag=f"Kf{b}")
                      for b in range(B)]
                Vraw = [[apool.tile([128, 512], BF16, name=f"Vr{b}_{blk}",
                                    tag=f"Vr{b}_{blk}") for blk in range(4)]
                        for b in range(B)]
                Vp = [[[apool.tile([128, 65], BF16, name=f"Vp{b}_{hh}_{kt}",
                                   tag=f"Vp{b}_{hh}_{kt}") for kt in range(16)]
                       for hh in range(2)] for b in range(B)]

                # rmsnorm1 sums first so the scale AllGather fires early and
                # hides under the qkv matmuls
                ssp = psA.tile([1, 512], F32, tag="psA")
                for k in range(8):
                    sq = work.tile([128, 512], BF16, tag="sq", bufs=2)
                    nc.scalar.activation(sq[:, :],
                                         xtmyA[:, k * 512:(k + 1) * 512],
                                         Act.Square)
                    nc.tensor.matmul(ssp[:, :], onesS[:, 0:1], sq[:, :],
                                     start=(k == 0), stop=(k == 7))
                lnm = work.tile([1, 512], F32, tag="lnm", bufs=1)
                nc.scalar.activation(lnm[:, :], ssp[:, :], Act.Ln,
                                     scale=1.0 / D, bias=epsS[0:1, :])
                srow = work.tile([1, 512], BF16, tag="srow", bufs=1)
                nc.scalar.activation(srow[:, :], lnm[:, :], Act.Exp, scale=-0.5)
                nc.gpsimd.dma_start(cin1[:, :], srow[:, :])
                nc.gpsimd.collective_compute(
                    "AllGather", Alu.bypass, [list(range(NC))],
                    ins=[cin1.opt()], outs=[cout1.opt()])
                sAG = wpool.tile([1, NC * 512], BF16, name="sAG", tag="sAG")
                nc.gpsimd.dma_start(sAG[0:1, :], cout1[:, :])

                # ---- qkv for BOTH batches before attention ----
                for b in range(B):
                    for blk in range(4):
                        j = 4 * b + blk
                        xta = apool.tile([128, 8 * 512], BF16, name="xta",
                                         tag="xta", bufs=3)
                        nc.sync.dma_start(xta[:, :],
                                          xTt[:, :, j * 512:(j + 1) * 512])
                        for m in range(3):
                            om = psB.tile([128, 512], F32, tag="psB")
                            for k in range(8):
                                nc.tensor.matmul(
                                    om[:, :],
                                    qkvW[:, k * 384 + m * 128:k * 384 + (m + 1) * 128],
                                    xta[:, k * 512:(k + 1) * 512],
                                    start=(k == 0), stop=(k == 7))
                            if m == 0:
                                nc.scalar.copy(Qf[b][:, blk * 512:(blk + 1) * 512],
                                               om[:, :])
                            elif m == 1:
                                nc.scalar.copy(Kf[b][:, blk * 512:(blk + 1) * 512],
                                               om[:, :])
                            else:
                                nc.scalar.copy(Vraw[b][blk][:, :], om[:, :])

                # ---- apply the scales; build V^T tiles ----
                for b in range(B):
                    for blk in range(4):
                        j = 4 * b + blk
                        rsp = psA.tile([128, 512], F32, tag="psA")
                        nc.tensor.matmul(rsp[:, :], onesS[0:1, 0:128],
                                         sAG[0:1, j * 512:(j + 1) * 512],
                                         start=True, stop=True)
                        rsbS = apool.tile([128, 512], dt, tag="rsb", bufs=2)
                        nc.scalar.copy(rsbS[:, :], rsp[:, :])
                        cs = slice(blk * 512, (blk + 1) * 512)
                        nc.vector.tensor_mul(Qf[b][:, cs], Qf[b][:, cs], rsbS[:, :])
                        nc.vector.tensor_mul(Kf[b][:, cs], Kf[b][:, cs], rsbS[:, :])
                        vfb = apool.tile([128, 512], BF16, tag="vfb", bufs=2)
                        nc.vector.tensor_mul(vfb[:, :], Vraw[b][blk][:, :],
                                             rsbS[:, :])
                        for sub in range(4):
                            kt = blk * 4 + sub
                            for hh in range(2):
                                vtp = psC.tile([128, 64], BF16, tag="psC")
                                nc.tensor.transpose(
                                    vtp[:, :],
                                    vfb[64 * hh:64 * hh + 64,
                                        sub * 128:(sub + 1) * 128],
                                    identS[64 * hh:64 * hh + 64,
                                           64 * hh:64 * hh + 64])
                                nc.vector.tensor_copy(Vp[b][hh][kt][:, 0:64],
                                                      vtp[:, :])
                                nc.vector.tensor_copy(Vp[b][hh][kt][:, 64:65],
                                                      onesS[:, 0:1])

                # ---- attention ----
                for b in range(B):
                    for hh in range(2):
                        r0 = 64 * hh
                        for qb in range(4):
                            q0 = qb * 512
                            ops = psC.tile([65, 512], F32, tag="psC")
                            nkt = 4 * qb + 4

                            def score_mm(kt):
                                # causal: queries before the key block never
                                # read these scores, so skip those columns
                                c0 = 128 * max(0, kt - 4 * qb)
                                sp = psB.tile([128, 512], F32, tag="psB")
                                nc.tensor.matmul(
                                    sp[:, c0:512],
                                    Kf[b][r0:r0 + 64, kt * 128:(kt + 1) * 128],
                                    Qf[b][r0:r0 + 64, q0 + c0:q0 + 512],
                                    start=True, stop=True)
                                return sp

                            # 2-deep score prefetch: keeps the exp latency off
                            # the PE critical path (psB ring is 3 banks)
                            spq = [score_mm(kt) for kt in range(min(2, nkt))]
                            for kt in range(nkt):
                                sp = spq.pop(0)
                                if kt + 2 < nkt:
                                    spq.append(score_mm(kt + 2))
                                e = apool.tile([128, 512], BF16, tag="expst",
                                               bufs=4)
                                d = kt - 4 * qb
                                if d < 0:
                                    nc.scalar.activation(e[:, :], sp[:, :],
                                                         Act.Exp, scale=0.125)
                                    nc.tensor.matmul(ops[:, :],
                                                     Vp[b][hh][kt][:, :],
                                                     e[:, :], start=(kt == 0),
                                                     stop=False)
                                else:
                                    nc.scalar.activation(e[:, 128 * d:512],
                                                         sp[:, 128 * d:512],
                                                         Act.Exp, scale=0.125)
                                    nc.vector.tensor_mul(
                                        e[:, 128 * d:128 * (d + 1)],
                                        e[:, 128 * d:128 * (d + 1)], triS[:, :])
                                    nc.tensor.matmul(ops[:, 128 * d:512],
                                                     Vp[b][hh][kt][:, :],
                                                     e[:, 128 * d:512],
                                                     start=(kt == 0),
                                                     stop=(kt == nkt - 1),
                                                     skip_group_check=True)
                            # 1/denominator = exp(-ln(den)) on Act engine
                            lnd = apool.tile([1, 512], F32, tag="lnd", bufs=2)
                            nc.scalar.activation(lnd[:, :], ops[64:65, :], Act.Ln)
                            rle = apool.tile([1, 512], BF16, tag="rle", bufs=2)
                            nc.scalar.activation(rle[:, :], lnd[:, :], Act.Exp,
                                                 scale=-1.0)
                            rb = psA.tile([64, 512], F32, tag="psA")
                            nc.tensor.matmul(rb[:, :], onesS[0:1, 0:64],
                                             rle[:, :], start=True, stop=True)
                            rbc = apool.tile([64, 512], dt, tag="rbc", bufs=2)
                            nc.scalar.copy(rbc[:, :], rb[:, :])
                            rbs = apool.tile([64, 512], BF16, tag="rbs", bufs=2)
                            nc.vector.tensor_mul(rbs[:, :], ops[0:64, :],
                                                 rbc[:, :])
                            nc.sync.dma_start(cin[4 * b + qb, r0:r0 + 64, :],
                                              rbs[:, :])

            # post-phase weights: issue now, overlapping the A2A
            nc.sync.dma_start(inW[:, :], inWt[:, :])
            nc.sync.dma_start(gateW[:, :], gateWt[:, :])
            nc.sync.dma_start(outW[:, :], out_wT[:, :])
            nc.sync.dma_start(dtW[:, :], dt_wT[:, :])
            nc.sync.dma_start(BpS[:, :], BpT[:, :])
            nc.sync.dma_start(CpS[:, :], CpT[:, :])

            nc.gpsimd.collective_compute(
                "AllToAll", Alu.bypass, [list(range(NC))],
                ins=[cin.opt()], outs=[cout.opt()])

            with tc.tile_pool(name="xpool", bufs=1) as xpool:
                x1d = [xpool.tile([128, 512], dt, name=f"x1d{m}", tag=f"x1d{m}")
                       for m in range(8)]
                h2T = [xpool.tile([128, 512], BF16, name=f"h2T{k}", tag=f"h2T{k}")
                       for k in range(8)]

                with tc.tile_pool(name="postA", bufs=1) as pA:
                    # attention rows for my tokens, loaded once (bf16)
                    otg = [pA.tile([128, 512], BF16, name=f"otg{k}", tag=f"otg{k}")
                           for k in range(8)]
                    for k in range(8):
                        nc.sync.dma_start(otg[k][:, :], cout[k, :, :])
                    # o_proj + residual (D-major)
                    for m in range(8):
                        pr = psB.tile([128, 512], F32, tag="psB")
                        for k in range(8):
                            nc.tensor.matmul(
                                pr[:, :],
                                oW[:, k * D + m * 128:k * D + (m + 1) * 128],
                                otg[k][:, :], start=(k == 0), stop=(k == 7))
                        nc.vector.tensor_add(x1d[m][:, :], pr[:, :],
                                             xtmyA[:, m * 512:(m + 1) * 512])

                # ---- rmsnorm2 (D-major) ----
                ssp2 = psA.tile([1, 512], F32, tag="psA")
                for k in range(8):
                    sq2 = work.tile([128, 512], BF16, tag="sq", bufs=2)
                    nc.scalar.activation(sq2[:, :], x1d[k][:, :], Act.Square)
                    nc.tensor.matmul(ssp2[:, :], onesS[:, 0:1], sq2[:, :],
                                     start=(k == 0), stop=(k == 7))
                ln2 = work.tile([1, 512], F32, tag="lnm", bufs=1)
                nc.scalar.activation(ln2[:, :], ssp2[:, :], Act.Ln,
                                     scale=1.0 / D, bias=epsS[0:1, :])
                s2row = work.tile([1, 512], BF16, tag="srow2", bufs=1)
                nc.scalar.activation(s2row[:, :], ln2[:, :], Act.Exp, scale=-0.5)
                rs2p = psA.tile([128, 512], F32, tag="psA")
                nc.tensor.matmul(rs2p[:, :], onesS[0:1, 0:128], s2row[:, :],
                                 start=True, stop=True)
                rsb2S = work.tile([128, 512], dt, tag="rsb2", bufs=1)
                nc.scalar.copy(rsb2S[:, :], rs2p[:, :])
                for k in range(8):
                    nc.vector.tensor_mul(h2T[k][:, :], x1d[k][:, :], rsb2S[:, :])

                with tc.tile_pool(name="postB", bufs=1) as pB:
                    # ---- scan projections (bf16 matmuls) ----
                    pz = psB.tile([SC, 512], F32, tag="psB")
                    for k in range(8):
                        nc.tensor.matmul(pz[:, :], inW[:, k * SC:(k + 1) * SC],
                                         h2T[k][:, :], start=(k == 0), stop=(k == 7))
                    z_s = pB.tile([SC, 512], BF16, name="z_s", tag="z_s")
                    nc.vector.tensor_copy(z_s[:, :], pz[:, :])
                    pdt = psB.tile([SC, 512], F32, tag="psB")
                    nc.tensor.matmul(pdt[:, :], dtW[:, :], z_s[:, :],
                                     start=True, stop=True)
                    dt_s = pB.tile([SC, 512], BF16, name="dt_s", tag="dt_s")
                    nc.scalar.activation(dt_s[:, :], pdt[:, :], Act.Exp,
                                         bias=dtbS[:, :])
                    nc.scalar.activation(dt_s[:, :], dt_s[:, :], Act.Ln, bias=1.0)
                    dtz_s = pB.tile([SC, 512], BF16, name="dtz_s", tag="dtz_s")
                    nc.vector.tensor_mul(dtz_s[:, :], dt_s[:, :], z_s[:, :])
                    pbi = psB.tile([ST, 512], F32, tag="psB")
                    nc.tensor.matmul(pbi[:, :], BpS[:, :], z_s[:, :],
                                     start=True, stop=True)
                    bi_s = pB.tile([ST, 512], BF16, name="bi_s", tag="bi_s")
                    nc.vector.tensor_copy(bi_s[:, :], pbi[:, :])
                    pci = psB.tile([ST, 512], F32, tag="psB")
                    nc.tensor.matmul(pci[:, :], CpS[:, :], z_s[:, :],
                                     start=True, stop=True)
                    ci_s = pB.tile([ST, 512], BF16, name="ci_s", tag="ci_s")
                    nc.vector.tensor_copy(ci_s[:, :], pci[:, :])

                    # ---- local scans (zero-init trajectories + cumprods) ----
                    ppT = [pB.tile([128, 512], BF16, name=f"ppT{g}", tag=f"ppT{g}")
                           for g in range(8)]
                    sc0T = [pB.tile([128, 512], dt, name=f"sc0T{g}",
                                    tag=f"sc0T{g}") for g in range(8)]
                    stg2 = pB.tile([128, 16], F32, name="stg2", tag="stg2")
                    for g in range(8):
                        pde = psB.tile([128, 512], F32, tag="psB")
                        nc.tensor.matmul(pde[:, :], escS[:, g * 128:(g + 1) * 128],
                                         dt_s[:, :], start=True, stop=True)
                        abar = pB.tile([128, 512], dt, tag="abar", bufs=2)
                        nc.scalar.activation(abar[:, :], pde[:, :], Act.Identity,
                                             scale=negA[:, g:g + 1], bias=1.0)
                        pdz = psB.tile([128, 512], F32, tag="psB")
                        nc.tensor.matmul(pdz[:, :], escS[:, g * 128:(g + 1) * 128],
                                         dtz_s[:, :], start=True, stop=True)
                        pbe = psC.tile([128, 512], F32, tag="psC")
                        nc.tensor.matmul(pbe[:, :], estS[:, :], bi_s[:, :],
                                         start=True, stop=True)
                        bes = pB.tile([128, 512], dt, tag="bes", bufs=2)
                        nc.scalar.copy(bes[:, :], pbe[:, :])
                        bin_ = pB.tile([128, 512], dt, tag="bin_", bufs=2)
                        nc.vector.tensor_mul(bin_[:, :], pdz[:, :], bes[:, :])
                        nc.vector.tensor_tensor_scan(sc0T[g][:, :], abar[:, :],
                                                     bin_[:, :], 0.0,
                                                     Alu.mult, Alu.add)
                        nc.vector.tensor_tensor_scan(ppT[g][:, :], abar[:, :],
                                                     abar[:, :], 1.0,
                                                     Alu.mult, Alu.bypass)
                        nc.vector.tensor_copy(stg2[:, g:g + 1],
                                              ppT[g][:, 511:512])
                        nc.vector.tensor_copy(stg2[:, 8 + g:8 + g + 1],
                                              sc0T[g][:, 511:512])

                    nc.gpsimd.dma_start(cin2[:, :], stg2[:, :])
                    nc.gpsimd.collective_compute(
                        "AllGather", Alu.bypass, [list(range(NC))],
                        ins=[cin2.opt()], outs=[cout2.opt()])

                    # overlap the AllGather with work that doesn't need it:
                    # C embedding and the gate (single act-table switch to Silu)
                    ces = pB.tile([128, 512], BF16, name="ces", tag="ces")
                    pce = psC.tile([128, 512], F32, tag="psC")
                    nc.tensor.matmul(pce[:, :], estS[:, :], ci_s[:, :],
                                     start=True, stop=True)
                    nc.vector.tensor_copy(ces[:, :], pce[:, :])
                    pg = psB.tile([SC, 512], F32, tag="psB")
                    for k in range(8):
                        nc.tensor.matmul(pg[:, :], gateW[:, k * SC:(k + 1) * SC],
                                         h2T[k][:, :], start=(k == 0), stop=(k == 7))
                    gate_s = pB.tile([SC, 512], BF16, name="gate_s", tag="gate_s")
                    nc.scalar.activation(gate_s[:, :], pg[:, :], Act.Silu)
                    # y = sc0*ces + sin*(pp*ces): precompute both products
                    y0T = [pB.tile([128, 512], BF16, name=f"y0T{g}", tag=f"y0T{g}")
                           for g in range(8)]
                    pcT = [pB.tile([128, 512], BF16, name=f"pcT{g}", tag=f"pcT{g}")
                           for g in range(8)]
                    for g in range(8):
                        nc.vector.tensor_mul(y0T[g][:, :], sc0T[g][:, :],
                                             ces[:, :])
                        nc.vector.tensor_mul(pcT[g][:, :], ppT[g][:, :],
                                             ces[:, :])

                    # ---- stitch initial states from preceding cores ----
                    pjs = []
                    for jj in range(NC):
                        pj = work.tile([128, 16], F32, tag=f"pj{jj}", bufs=1)
                        nc.sync.dma_start(pj[:, :],
                                          cout2[jj * 128:(jj + 1) * 128, :])
                        pjs.append(pj)
                    sin = pB.tile([128, 8], F32, name="sin", tag="sin")
                    nc.vector.memset(sin[:, :], 0.0)
                    for jj in range(NC):
                        pe_ = work.tile([128, 8], F32, tag="pe_")
                        nc.vector.scalar_tensor_tensor(pe_[:, :], pjs[jj][:, 0:8],
                                                       cselS[:, jj:jj + 1],
                                                       omcS[:, 8 * jj:8 * jj + 8],
                                                       Alu.mult, Alu.add)
                        se_ = work.tile([128, 8], F32, tag="se_")
                        nc.vector.tensor_scalar_mul(se_[:, :], pjs[jj][:, 8:16],
                                                    cselS[:, jj:jj + 1])
                        nc.vector.tensor_mul(sin[:, :], sin[:, :], pe_[:, :])
                        nc.vector.tensor_add(sin[:, :], sin[:, :], se_[:, :])

                    # ---- finalize: y_g = y0 + sin_g * pc ----
                    py = psA.tile([SC, 512], F32, tag="psA")
                    for g in range(8):
                        yt = pB.tile([128, 512], BF16, tag="yt", bufs=2)
                        nc.vector.scalar_tensor_tensor(yt[:, :], pcT[g][:, :],
                                                       sin[:, g:g + 1],
                                                       y0T[g][:, :],
                                                       Alu.mult, Alu.add)
                        nc.tensor.matmul(py[:, :], r8S[:, g * 64:(g + 1) * 64],
                                         yt[:, :], start=(g == 0), stop=(g == 7))
                    yT = pB.tile([SC, 512], dt, name="yT", tag="yT")
                    nc.vector.tensor_copy(yT[:, :], py[:, :])

                    # ---- gate + out_proj + final residual (D-major) ----
                    yg = pB.tile([SC, 512], BF16, name="yg", tag="yg")
                    nc.vector.tensor_mul(yg[:, :], yT[:, :], gate_s[:, :])
                    for m in range(8):
                        p2 = psB.tile([128, 512], F32, tag="psB")
                        nc.tensor.matmul(p2[:, :], outW[:, m * 128:(m + 1) * 128],
                                         yg[:, :], start=True, stop=True)
                        yo = pB.tile([128, 512], dt, tag="yo", bufs=2)
                        nc.vector.tensor_add(yo[:, :], p2[:, :], x1d[m][:, :])
                        nc.sync.dma_start(yout[m * 128:(m + 1) * 128, :], yo[:, :])

    _split_multi_waits(nc)
    return nc


def kernel(x, qkv_w, o_w, norm1_w, norm2_w, in_w, out_w, A_log, Bp_w, Cp_w,
           dt_w, dt_b, gate_w):
    import ml_dtypes
    f = np.float32
    bf = ml_dtypes.bfloat16
    xf = np.ascontiguousarray(np.asarray(x, f).reshape(BT, D))
    xT_f = np.ascontiguousarray(xf.T)
    # pre-tiled x: [128, k, t] with row-block k of xT in column group k
    xTt = np.ascontiguousarray(
        xT_f.reshape(8, 128, BT).transpose(1, 0, 2)).astype(bf)
    # fold the rmsnorm elementwise weights into the consuming projections
    qkv_w1 = np.asarray(qkv_w, f) * np.asarray(norm1_w, f)[None, :]
    in_w2 = np.asarray(in_w, f) * np.asarray(norm2_w, f)[None, :]
    gate_w2 = np.asarray(gate_w, f) * np.asarray(norm2_w, f)[None, :]

    def tile128(wT):  # [D, W] -> [128, 8*W] with row-block k at column group k
        Dd, W = wT.shape
        return np.ascontiguousarray(
            wT.reshape(8, 128, W).transpose(1, 0, 2).reshape(128, 8 * W))

    oWt = tile128(np.asarray(o_w, f).T).astype(bf)
    inWt = tile128(in_w2.T).astype(bf)
    gateWt = tile128(gate_w2.T).astype(bf)
    out_wT = np.ascontiguousarray(np.asarray(out_w, f).T.astype(bf))
    dt_wT = np.ascontiguousarray(np.asarray(dt_w, f).T.astype(bf))
    BpT = np.ascontiguousarray(np.asarray(Bp_w, f).T.astype(bf))
    CpT = np.ascontiguousarray(np.asarray(Cp_w, f).T.astype(bf))
    dtbv = np.ascontiguousarray(np.asarray(dt_b, f).reshape(SC, 1))
    alogv = np.ascontiguousarray(np.asarray(A_log, f).reshape(1024).reshape(8, 128).T)
    ident = np.eye(128, dtype=f)
    tri_m = (np.arange(128)[None, :] >= np.arange(128)[:, None]).astype(f)
    cpkB = np.ascontiguousarray(
        np.concatenate([ident, tri_m, np.ones((128, 128), f)], axis=1).astype(bf))
    ejm = np.zeros((8, 1024), f)
    for j in range(8):
        ejm[j, j * 128:(j + 1) * 128] = 1.0
    jj = np.arange(1024)
    escm = (np.arange(SC)[:, None] == (jj[None, :] // 16)).astype(bf)
    estm = (np.arange(ST)[:, None] == (np.arange(128)[None, :] % 16)).astype(bf)
    r8m = np.zeros((128, 512), f)
    for g in range(8):
        for j in range(128):
            r8m[j, g * 64 + 8 * g + j // 16] = 1.0
    r8m = r8m.astype(bf)

    nc = _build()
    in_maps = []
    for c in range(NC):
        b, q = c // 4, c % 4
        h0 = 2 * c
        rows = np.concatenate([np.arange(h0 * 64, (h0 + 2) * 64),
                               D + np.arange(h0 * 64, (h0 + 2) * 64),
                               2 * D + np.arange(h0 * 64, (h0 + 2) * 64)])
        qkvWt = tile128(qkv_w1[rows, :].T).astype(bf)
        xmyT2 = np.ascontiguousarray(
            xT_f[:, c * TOK:(c + 1) * TOK].reshape(8, 128, TOK)
            .transpose(1, 0, 2).reshape(128, 8 * TOK))
        sel = np.zeros(NC, f)
        for j in range(q):
            sel[4 * b + j] = 1.0
        cselv = np.ascontiguousarray(np.tile(sel[None, :], (128, 1)))
        omcv = np.ascontiguousarray(
            np.repeat(1.0 - sel, 8)[None, :].repeat(128, axis=0).astype(f))
        in_maps.append({
            "xTt": xTt, "xmyT2": xmyT2, "qkvWt": qkvWt, "oWt": oWt,
            "inWt": inWt, "gateWt": gateWt, "out_wT": out_wT,
            "dt_wT": dt_wT, "BpT": BpT, "CpT": CpT,
            "dtb": dtbv, "alog": alogv, "cpkB": cpkB, "ej": ejm,
            "esc": escm, "est": estm, "r8": r8m,
            "csel": cselv, "omc": omcv,
            "epsb": np.full((128, 1), EPS, f),
        })
    import os
    trace = bool(int(os.environ.get("BASS_PROFILE", "0")))
    res = run_bass_kernel_spmd(nc, in_maps, core_ids=list(range(NC)),
                               trace=trace)
    if trace:
        print("HW exec time:", res.exec_time_ns, "ns")
        print("trace:", res.instructions_and_trace[1] if res.instructions_and_trace else None)
    out = np.concatenate([res.results[c]["yout"].T for c in range(NC)], axis=0)
    return out.reshape(B, T, D)


# revision 35
# speedup vs baseline: 1.0014x; 1.0014x over previous
import numpy as np
import concourse.bass as bass
import concourse.mybir as mybir
from concourse.bass_utils import run_bass_kernel_spmd
from concourse.tile import TileContext
from concourse.mybir import AluOpType as Alu, ActivationFunctionType as Act

B, T, D, H, hd, SC, ST = 2, 2048, 1024, 16, 64, 64, 16
BT = B * T          # 4096
NC = 8
TOK = BT // NC      # 512 tokens per core
EPS = 1.1920929e-07
F32 = mybir.dt.float32
BF16 = mybir.dt.bfloat16


def _split_multi_waits(nc, max_waits=1):
    # this walrus build accepts only one sync wait per ISA instruction
    n = 0
    for f in nc.m.functions:
        for bb in f.blocks:
            out = []
            for inst in bb.instructions:
                si = inst.sync_info
                if si is not None and si.on_wait and len(si.on_wait) > max_waits:
                    for w in si.on_wait[:-max_waits]:
                        out.append(mybir.InstNoOp(
                            name=f"{inst.name}_ws{n}", ins=[], outs=[],
                            engine=inst.engine,
                            sync_info=mybir.SyncInfo(on_wait=[w], on_update=[]),
                            bass_nofuse=True))
                        n += 1
                    inst.sync_info = mybir.SyncInfo(
                        on_wait=si.on_wait[-max_waits:], on_update=si.on_update)
                out.append(inst)
            bb.instructions = out
    return n


def _build():
    nc = bass.Bass()
    dt = mybir.dt.float32r if int(__import__("os").environ.get("BASS_F32R", "1")) else F32

    # pre-tiled inputs: [128, k*W+j] layouts so each stream is one DMA
    xTt = nc.dram_tensor("xTt", [128, 8, BT], BF16, kind="ExternalInput")
    xmyT2 = nc.dram_tensor("xmyT2", [128, 8 * TOK], dt, kind="ExternalInput")
    qkvWt = nc.dram_tensor("qkvWt", [128, 8 * 384], BF16, kind="ExternalInput")
    oWt = nc.dram_tensor("oWt", [128, 8 * D], BF16, kind="ExternalInput")
    inWt = nc.dram_tensor("inWt", [128, 8 * SC], BF16, kind="ExternalInput")
    gateWt = nc.dram_tensor("gateWt", [128, 8 * SC], BF16, kind="ExternalInput")
    out_wT = nc.dram_tensor("out_wT", [SC, D], BF16, kind="ExternalInput")
    dt_wT = nc.dram_tensor("dt_wT", [SC, SC], BF16, kind="ExternalInput")
    BpT = nc.dram_tensor("BpT", [SC, ST], BF16, kind="ExternalInput")
    CpT = nc.dram_tensor("CpT", [SC, ST], BF16, kind="ExternalInput")
    dtb = nc.dram_tensor("dtb", [SC, 1], F32, kind="ExternalInput")
    alog = nc.dram_tensor("alog", [128, 8], F32, kind="ExternalInput")
    cpkB = nc.dram_tensor("cpkB", [128, 384], BF16, kind="ExternalInput")
    ej = nc.dram_tensor("ej", [8, 1024], dt, kind="ExternalInput")
    esc = nc.dram_tensor("esc", [SC, 1024], BF16, kind="ExternalInput")
    est = nc.dram_tensor("est", [ST, 128], BF16, kind="ExternalInput")
    r8 = nc.dram_tensor("r8", [128, 512], BF16, kind="ExternalInput")
    csel = nc.dram_tensor("csel", [128, 8], F32, kind="ExternalInput")
    omc = nc.dram_tensor("omc", [128, 64], F32, kind="ExternalInput")
    epsb = nc.dram_tensor("epsb", [128, 1], F32, kind="ExternalInput")

    yout = nc.dram_tensor("yout", [D, TOK], dt, kind="ExternalOutput")

    with nc.allow_low_precision(reason="bf16 matmul paths; f32r is fp32 bytes"), \
         TileContext(nc) as tc:
        with tc.tile_pool(name="const", bufs=1) as cpool, \
             tc.tile_pool(name="wts", bufs=1) as wpool, \
             tc.tile_pool(name="ow", bufs=1) as owp, \
             tc.tile_pool(name="xmy", bufs=1) as xmp, \
             tc.tile_pool(name="work", bufs=2) as work, \
             tc.tile_pool(name="psA", bufs=2, space="PSUM") as psA, \
             tc.tile_pool(name="psB", bufs=3, space="PSUM") as psB, \
             tc.tile_pool(name="psC", bufs=2, space="PSUM") as psC, \
             tc.tile_pool(name="dram", bufs=1, space="DRAM") as dram:

            # ---- highest-priority DMAs first (single triggers each) ----
            xtmyA = xmp.tile([128, 8 * TOK], dt, name="xtmyA", tag="xtmyA")
            nc.sync.dma_start(xtmyA[:, :], xmyT2[:, :])

            # packed bf16 consts: ident | tri | ones
            cpkS = cpool.tile([128, 384], BF16, name="cpkS", tag="cpkS")
            nc.sync.dma_start(cpkS[:, :], cpkB[:, :])
            identS = cpkS[:, 0:128]
            triS = cpkS[:, 128:256]
            onesS = cpkS[:, 256:384]

            def csbuf(shape, src, name, d=None):
                t = cpool.tile(shape, d or dt, name=name, tag=name)
                nc.sync.dma_start(t[:, :], src)
                return t

            ejS = csbuf([8, 1024], ej[:, :], "ejS")
            escS = csbuf([SC, 1024], esc[:, :], "escS", BF16)
            estS = csbuf([ST, 128], est[:, :], "estS", BF16)
            r8S = csbuf([128, 512], r8[:, :], "r8S", BF16)
            cselS = csbuf([128, 8], csel[:, :], "cselS", F32)
            omcS = csbuf([128, 64], omc[:, :], "omcS", F32)
            dtbS = csbuf([SC, 1], dtb[:, :], "dtbS", F32)
            alogS = csbuf([128, 8], alog[:, :], "alogS", F32)
            epsS = csbuf([128, 1], epsb[:, :], "epsS", F32)

            zerosF = cpool.tile([128, 512], F32, name="zerosF", tag="zerosF")
            nc.vector.memset(zerosF[:, :], 0.0)
            negA = cpool.tile([128, 8], F32, name="negA", tag="negA")
            nc.scalar.activation(negA[:, :], alogS[:, :], Act.Exp)
            nc.vector.tensor_scalar_mul(negA[:, :], negA[:, :], -1.0)

            # persistent post-phase weight tiles (DMAs issued later)
            inW = wpool.tile([128, 8 * SC], BF16, name="inW", tag="inW")
            gateW = wpool.tile([128, 8 * SC], BF16, name="gateW", tag="gateW")
            outW = wpool.tile([SC, D], BF16, name="outW", tag="outW")
            dtW = wpool.tile([SC, SC], BF16, name="dtW", tag="dtW")
            BpS = wpool.tile([SC, ST], BF16, name="BpS", tag="BpS")
            CpS = wpool.tile([SC, ST], BF16, name="CpS", tag="CpS")
            # o_proj weights on the gpsimd queue (one trigger)
            oW = owp.tile([128, 8 * D], BF16, name="oW", tag="oW")
            nc.gpsimd.dma_start(oW[:, :], oWt[:, :])

            # dram collective buffers
            cin1 = dram.tile([1, 512], BF16, name="cin1T", tag="cin1T")
            cout1 = dram.tile([NC, 512], BF16, name="cout1T", tag="cout1T")
            cin = dram.tile([NC, 128, TOK], BF16, name="cinT", tag="cinT")
            cout = dram.tile([NC, 128, TOK], BF16, name="coutT", tag="coutT")
            cin2 = dram.tile([128, 16], F32, name="cin2T", tag="cin2T")
            cout2 = dram.tile([NC * 128, 16], F32, name="cout2T", tag="cout2T")

            with tc.tile_pool(name="attn", bufs=1) as apool:
                qkvW = apool.tile([128, 8 * 384], BF16, name="qkvW", tag="qkvW")
                nc.sync.dma_start(qkvW[:, :], qkvWt[:, :])
                Qf = [apool.tile([128, T], BF16, name=f"Qf{b}", tag=f"Qf{b}")
                      for b in range(B)]
                # K stored zero-padded per head: Kz[b][hh] keeps head hh's 64
                # rows in place and zeros the other head's rows, so score
                # matmuls contract over K=128 (full-rate on the PE)
                Kz = [[apool.tile([128, T], BF16, name=f"Kz{b}_{hh}",
                                  tag=f"Kz{b}_{hh}") for hh in range(2)]
                      for b in range(B)]
                Vraw = [[apool.tile([128, 512], BF16, name=f"Vr{b}_{blk}",
                                    tag=f"Vr{b}_{blk}") for blk in range(4)]
                        for b in range(B)]
                # V^T tiles padded to 128 output columns (full-rate M=128):
                # cols 0-63 = V^T, col 64 = ones (denominator), 65-127 = zeros
                Vp = [[[apool.tile([128, 128], BF16, name=f"Vp{b}_{hh}_{kt}",
                                   tag=f"Vp{b}_{hh}_{kt}") for kt in range(16)]
                       for hh in range(2)] for b in range(B)]
                for b in range(B):
                    for hh in range(2):
                        z0, z1 = (64, 128) if hh == 0 else (0, 64)
                        for cc in range(4):
                            nc.vector.tensor_copy(
                                Kz[b][hh][z0:z1, cc * 512:(cc + 1) * 512],
                                zerosF[z0:z1, :])
                        for kt in range(16):
                            nc.vector.tensor_copy(Vp[b][hh][kt][:, 65:128],
                                                  zerosF[:, 0:63])

                # rmsnorm1 sums first so the scale AllGather fires early and
                # hides under the qkv matmuls
                ssp = psA.tile([1, 512], F32, tag="psA")
                for k in range(8):
                    sq = work.tile([128, 512], BF16, tag="sq", bufs=2)
                    nc.scalar.activation(sq[:, :],
                                         xtmyA[:, k * 512:(k + 1) * 512],
                                         Act.Square)
                    nc.tensor.matmul(ssp[:, :], onesS[:, 0:1], sq[:, :],
                                     start=(k == 0), stop=(k == 7))
                lnm = work.tile([1, 512], F32, tag="lnm", bufs=1)
                nc.scalar.activation(lnm[:, :], ssp[:, :], Act.Ln,
                                     scale=1.0 / D, bias=epsS[0:1, :])
                srow = work.tile([1, 512], BF16, tag="srow", bufs=1)
                nc.scalar.activation(srow[:, :], lnm[:, :], Act.Exp, scale=-0.5)
                nc.gpsimd.dma_start(cin1[:, :], srow[:, :])
                nc.gpsimd.collective_compute(
                    "AllGather", Alu.bypass, [list(range(NC))],
                    ins=[cin1.opt()], outs=[cout1.opt()])
                sAG = wpool.tile([1, NC * 512], BF16, name="sAG", tag="sAG")
                nc.gpsimd.dma_start(sAG[0:1, :], cout1[:, :])

                # ---- qkv for BOTH batches before attention ----
                for b in range(B):
                    for blk in range(4):
                        j = 4 * b + blk
                        xta = apool.tile([128, 8 * 512], BF16, name="xta",
                                         tag="xta", bufs=3)
                        nc.sync.dma_start(xta[:, :],
                                          xTt[:, :, j * 512:(j + 1) * 512])
                        for m in range(3):
                            om = psB.tile([128, 512], F32, tag="psB")
                            for k in range(8):
                                nc.tensor.matmul(
                                    om[:, :],
                                    qkvW[:, k * 384 + m * 128:k * 384 + (m + 1) * 128],
                                    xta[:, k * 512:(k + 1) * 512],
                                    start=(k == 0), stop=(k == 7))
                            if m == 0:
                                nc.scalar.copy(Qf[b][:, blk * 512:(blk + 1) * 512],
                                               om[:, :])
                            elif m == 1:
                                cs_ = slice(blk * 512, (blk + 1) * 512)
                                nc.scalar.copy(Kz[b][0][0:64, cs_], om[0:64, :])
                                nc.scalar.copy(Kz[b][1][64:128, cs_],
                                               om[64:128, :])
                            else:
                                nc.scalar.copy(Vraw[b][blk][:, :], om[:, :])

                # ---- apply the scales; build V^T tiles ----
                for b in range(B):
                    for blk in range(4):
                        j = 4 * b + blk
                        rsp = psA.tile([128, 512], F32, tag="psA")
                        nc.tensor.matmul(rsp[:, :], onesS[0:1, 0:128],
                                         sAG[0:1, j * 512:(j + 1) * 512],
                                         start=True, stop=True)
                        rsbS = apool.tile([128, 512], dt, tag="rsb", bufs=2)
                        nc.scalar.copy(rsbS[:, :], rsp[:, :])
                        cs = slice(blk * 512, (blk + 1) * 512)
                        nc.vector.tensor_mul(Qf[b][:, cs], Qf[b][:, cs], rsbS[:, :])
                        nc.vector.tensor_mul(Kz[b][0][0:64, cs], Kz[b][0][0:64, cs],
                                             rsbS[0:64, :])
                        nc.vector.tensor_mul(Kz[b][1][64:128, cs],
                                             Kz[b][1][64:128, cs],
                                             rsbS[64:128, :])
                        vfb = apool.tile([128, 512], BF16, tag="vfb", bufs=2)
                        nc.vector.tensor_mul(vfb[:, :], Vraw[b][blk][:, :],
                                             rsbS[:, :])
                        for sub in range(4):
                            kt = blk * 4 + sub
                            for hh in range(2):
                                vtp = psC.tile([128, 64], BF16, tag="psC")
                                nc.tensor.transpose(
                                    vtp[:, :],
                                    vfb[64 * hh:64 * hh + 64,
                                        sub * 128:(sub + 1) * 128],
                                    identS[64 * hh:64 * hh + 64,
                                           64 * hh:64 * hh + 64])
                                nc.vector.tensor_copy(Vp[b][hh][kt][:, 0:64],
                                                      vtp[:, :])
                                nc.vector.tensor_copy(Vp[b][hh][kt][:, 64:65],
                                                      onesS[:, 0:1])

                # ---- attention ----
                for b in range(B):
                    for hh in range(2):
                        r0 = 64 * hh
                        for qb in range(4):
                            q0 = qb * 512
                            ops = psC.tile([128, 512], F32, tag="psC")
                            nkt = 4 * qb + 4

                            def score_mm(kt):
                                # causal: queries before the key block never
                                # read these scores, so skip those columns
                                c0 = 128 * max(0, kt - 4 * qb)
                                sp = psB.tile([128, 512], F32, tag="psB")
                                nc.tensor.matmul(
                                    sp[:, c0:512],
                                    Kz[b][hh][:, kt * 128:(kt + 1) * 128],
                                    Qf[b][:, q0 + c0:q0 + 512],
                                    start=True, stop=True)
                                return sp

                            # 2-deep score prefetch: keeps the exp latency off
                            # the PE critical path (psB ring is 3 banks)
                            spq = [score_mm(kt) for kt in range(min(2, nkt))]
                            for kt in range(nkt):
                                sp = spq.pop(0)
                                if kt + 2 < nkt:
                                    spq.append(score_mm(kt + 2))
                                e = apool.tile([128, 512], BF16, tag="expst",
                                               bufs=4)
                                d = kt - 4 * qb
                                if d < 0:
                                    nc.scalar.activation(e[:, :], sp[:, :],
                                                         Act.Exp, scale=0.125)
                                    nc.tensor.matmul(ops[:, :],
                                                     Vp[b][hh][kt][:, :],
                                                     e[:, :], start=(kt == 0),
                                                     stop=False)
                                else:
                                    nc.scalar.activation(e[:, 128 * d:512],
                                                         sp[:, 128 * d:512],
                                                         Act.Exp, scale=0.125)
                                    nc.vector.tensor_mul(
                                        e[:, 128 * d:128 * (d + 1)],
                                        e[:, 128 * d:128 * (d + 1)], triS[:, :])
                                    nc.tensor.matmul(ops[:, 128 * d:512],
                                                     Vp[b][hh][kt][:, :],
                                                     e[:, 128 * d:512],
                                                     start=(kt == 0),
                                                     stop=(kt == nkt - 1),
                                                     skip_group_check=True)
                            # 1/denominator = exp(-ln(den)) on Act engine
                            lnd = apool.tile([1, 512], F32, tag="lnd", bufs=2)
                            nc.scalar.activation(lnd[:, :], ops[64:65, :], Act.Ln)
                            rle = apool.tile([1, 512], BF16, tag="rle", bufs=2)
                            nc.scalar.activation(rle[:, :], lnd[:, :], Act.Exp,
                                                 scale=-1.0)
                            rb = psA.tile([64, 512], F32, tag="psA")
                            nc.tensor.matmul(rb[:, :], onesS[0:1, 0:64],
                                             rle[:, :], start=True, stop=True)
                            rbc = apool.tile([64, 512], dt, tag="rbc", bufs=2)
                            nc.vector.tensor_copy(rbc[:, :], rb[:, :])
                            rbs = apool.tile([64, 512], BF16, tag="rbs", bufs=2)
                            nc.vector.tensor_mul(rbs[:, :], ops[0:64, :],
                                                 rbc[:, :])
                            nc.sync.dma_start(cin[4 * b + qb, r0:r0 + 64, :],
                                              rbs[:, :])

            # post-phase weights: issue now, overlapping the A2A
            nc.sync.dma_start(inW[:, :], inWt[:, :])
            nc.sync.dma_start(gateW[:, :], gateWt[:, :])
            nc.sync.dma_start(outW[:, :], out_wT[:, :])
            nc.sync.dma_start(dtW[:, :], dt_wT[:, :])
            nc.sync.dma_start(BpS[:, :], BpT[:, :])
            nc.sync.dma_start(CpS[:, :], CpT[:, :])

            nc.gpsimd.collective_compute(
                "AllToAll", Alu.bypass, [list(range(NC))],
                ins=[cin.opt()], outs=[cout.opt()])

            with tc.tile_pool(name="xpool", bufs=1) as xpool:
                x1d = [xpool.tile([128, 512], dt, name=f"x1d{m}", tag=f"x1d{m}")
                       for m in range(8)]
                h2T = [xpool.tile([128, 512], BF16, name=f"h2T{k}", tag=f"h2T{k}")
                       for k in range(8)]

                with tc.tile_pool(name="postA", bufs=1) as pA:
                    # attention rows for my tokens, loaded once (bf16)
                    otg = [pA.tile([128, 512], BF16, name=f"otg{k}", tag=f"otg{k}")
                           for k in range(8)]
                    for k in range(8):
                        nc.sync.dma_start(otg[k][:, :], cout[k, :, :])
                    # o_proj + residual (D-major)
                    for m in range(8):
                        pr = psB.tile([128, 512], F32, tag="psB")
                        for k in range(8):
                            nc.tensor.matmul(
                                pr[:, :],
                                oW[:, k * D + m * 128:k * D + (m + 1) * 128],
                                otg[k][:, :], start=(k == 0), stop=(k == 7))
                        nc.vector.tensor_add(x1d[m][:, :], pr[:, :],
                                             xtmyA[:, m * 512:(m + 1) * 512])

                # ---- rmsnorm2 (D-major) ----
                ssp2 = psA.tile([1, 512], F32, tag="psA")
                for k in range(8):
                    sq2 = work.tile([128, 512], BF16, tag="sq", bufs=2)
                    nc.scalar.activation(sq2[:, :], x1d[k][:, :], Act.Square)
                    nc.tensor.matmul(ssp2[:, :], onesS[:, 0:1], sq2[:, :],
                                     start=(k == 0), stop=(k == 7))
                ln2 = work.tile([1, 512], F32, tag="lnm", bufs=1)
                nc.scalar.activation(ln2[:, :], ssp2[:, :], Act.Ln,
                                     scale=1.0 / D, bias=epsS[0:1, :])
                s2row = work.tile([1, 512], BF16, tag="srow2", bufs=1)
                nc.scalar.activation(s2row[:, :], ln2[:, :], Act.Exp, scale=-0.5)
                rs2p = psA.tile([128, 512], F32, tag="psA")
                nc.tensor.matmul(rs2p[:, :], onesS[0:1, 0:128], s2row[:, :],
                                 start=True, stop=True)
                rsb2S = work.tile([128, 512], dt, tag="rsb2", bufs=1)
                nc.scalar.copy(rsb2S[:, :], rs2p[:, :])
                for k in range(8):
                    nc.vector.tensor_mul(h2T[k][:, :], x1d[k][:, :], rsb2S[:, :])

                with tc.tile_pool(name="postB", bufs=1) as pB:
                    # ---- scan projections (bf16 matmuls) ----
                    pz = psB.tile([SC, 512], F32, tag="psB")
                    for k in range(8):
                        nc.tensor.matmul(pz[:, :], inW[:, k * SC:(k + 1) * SC],
                                         h2T[k][:, :], start=(k == 0), stop=(k == 7))
                    z_s = pB.tile([SC, 512], BF16, name="z_s", tag="z_s")
                    nc.vector.tensor_copy(z_s[:, :], pz[:, :])
                    pdt = psB.tile([SC, 512], F32, tag="psB")
                    nc.tensor.matmul(pdt[:, :], dtW[:, :], z_s[:, :],
                                     start=True, stop=True)
                    dt_s = pB.tile([SC, 512], BF16, name="dt_s", tag="dt_s")
                    nc.scalar.activation(dt_s[:, :], pdt[:, :], Act.Exp,
                                         bias=dtbS[:, :])
                    nc.scalar.activation(dt_s[:, :], dt_s[:, :], Act.Ln, bias=1.0)
                    dtz_s = pB.tile([SC, 512], BF16, name="dtz_s", tag="dtz_s")
                    nc.vector.tensor_mul(dtz_s[:, :], dt_s[:, :], z_s[:, :])
                    pbi = psB.tile([ST, 512], F32, tag="psB")
                    nc.tensor.matmul(pbi[:, :], BpS[:, :], z_s[:, :],
                                     start=True, stop=True)
                    bi_s = pB.tile([ST, 512], BF16, name="bi_s", tag="bi_s")
                    nc.vector.tensor_copy(bi_s[:, :], pbi[:, :])
                    pci = psB.tile([ST, 512], F32, tag="psB")
                    nc.tensor.matmul(pci[:, :], CpS[:, :], z_s[:, :],
                                     start=True, stop=True)
                    ci_s = pB.tile([ST, 512], BF16, name="ci_s", tag="ci_s")
                    nc.vector.tensor_copy(ci_s[:, :], pci[:, :])

                    # ---- local scans (zero-init trajectories + cumprods) ----
                    ppT = [pB.tile([128, 512], BF16, name=f"ppT{g}", tag=f"ppT{g}")
                           for g in range(8)]
                    sc0T = [pB.tile([128, 512], dt, name=f"sc0T{g}",
                                    tag=f"sc0T{g}") for g in range(8)]
                    stg2 = pB.tile([128, 16], F32, name="stg2", tag="stg2")
                    for g in range(8):
                        pde = psB.tile([128, 512], F32, tag="psB")
                        nc.tensor.matmul(pde[:, :], escS[:, g * 128:(g + 1) * 128],
                                         dt_s[:, :], start=True, stop=True)
                        abar = pB.tile([128, 512], dt, tag="abar", bufs=3)
                        nc.scalar.activation(abar[:, :], pde[:, :], Act.Identity,
                                             scale=negA[:, g:g + 1], bias=1.0)
                        pdz = psB.tile([128, 512], F32, tag="psB")
                        nc.tensor.matmul(pdz[:, :], escS[:, g * 128:(g + 1) * 128],
                                         dtz_s[:, :], start=True, stop=True)
                        pbe = psC.tile([128, 512], F32, tag="psC")
                        nc.tensor.matmul(pbe[:, :], estS[:, :], bi_s[:, :],
                                         start=True, stop=True)
                        bes = pB.tile([128, 512], dt, tag="bes", bufs=3)
                        nc.scalar.copy(bes[:, :], pbe[:, :])
                        bin_ = pB.tile([128, 512], dt, tag="bin_", bufs=3)
                        nc.vector.tensor_mul(bin_[:, :], pdz[:, :], bes[:, :])
                        nc.vector.tensor_tensor_scan(sc0T[g][:, :], abar[:, :],
                                                     bin_[:, :], 0.0,
                                                     Alu.mult, Alu.add)
                        nc.vector.tensor_tensor_scan(ppT[g][:, :], abar[:, :],
                                                     abar[:, :], 1.0,
                                                     Alu.mult, Alu.bypass)
                        nc.vector.tensor_copy(stg2[:, g:g + 1],
                                              ppT[g][:, 511:512])
                        nc.vector.tensor_copy(stg2[:, 8 + g:8 + g + 1],
                                              sc0T[g][:, 511:512])

                    nc.gpsimd.dma_start(cin2[:, :], stg2[:, :])
                    nc.gpsimd.collective_compute(
                        "AllGather", Alu.bypass, [list(range(NC))],
                        ins=[cin2.opt()], outs=[cout2.opt()])

                    # overlap the AllGather with work that doesn't need it:
                    # C embedding and the gate (single act-table switch to Silu)
                    ces = pB.tile([128, 512], BF16, name="ces", tag="ces")
                    pce = psC.tile([128, 512], F32, tag="psC")
                    nc.tensor.matmul(pce[:, :], estS[:, :], ci_s[:, :],
                                     start=True, stop=True)
                    nc.vector.tensor_copy(ces[:, :], pce[:, :])
                    pg = psB.tile([SC, 512], F32, tag="psB")
                    for k in range(8):
                        nc.tensor.matmul(pg[:, :], gateW[:, k * SC:(k + 1) * SC],
                                         h2T[k][:, :], start=(k == 0), stop=(k == 7))
                    gate_s = pB.tile([SC, 512], BF16, name="gate_s", tag="gate_s")
                    nc.scalar.activation(gate_s[:, :], pg[:, :], Act.Silu)
                    # y = sc0*ces + sin*(pp*ces): precompute both products
                    y0T = [pB.tile([128, 512], BF16, name=f"y0T{g}", tag=f"y0T{g}")
                           for g in range(8)]
                    pcT = [pB.tile([128, 512], BF16, name=f"pcT{g}", tag=f"pcT{g}")
                           for g in range(8)]
                    for g in range(8):
                        nc.vector.tensor_mul(y0T[g][:, :], sc0T[g][:, :],
                                             ces[:, :])
                        nc.vector.tensor_mul(pcT[g][:, :], ppT[g][:, :],
                                             ces[:, :])

                    # ---- stitch initial states from preceding cores ----
                    pjs = []
                    for jj in range(NC):
                        pj = work.tile([128, 16], F32, tag=f"pj{jj}", bufs=1)
                        nc.sync.dma_start(pj[:, :],
                                          cout2[jj * 128:(jj + 1) * 128, :])
                        pjs.append(pj)
                    sin = pB.tile([128, 8], F32, name="sin", tag="sin")
                    nc.vector.memset(sin[:, :], 0.0)
                    for jj in range(NC):
                        pe_ = work.tile([128, 8], F32, tag="pe_")
                        nc.vector.scalar_tensor_tensor(pe_[:, :], pjs[jj][:, 0:8],
                                                       cselS[:, jj:jj + 1],
                                                       omcS[:, 8 * jj:8 * jj + 8],
                                                       Alu.mult, Alu.add)
                        se_ = work.tile([128, 8], F32, tag="se_")
                        nc.vector.tensor_scalar_mul(se_[:, :], pjs[jj][:, 8:16],
                                                    cselS[:, jj:jj + 1])
                        nc.vector.tensor_mul(sin[:, :], sin[:, :], pe_[:, :])
                        nc.vector.tensor_add(sin[:, :], sin[:, :], se_[:, :])

                    # ---- finalize: y_g = y0 + sin_g * pc ----
                    py = psA.tile([SC, 512], F32, tag="psA")
                    for g in range(8):
                        yt = pB.tile([128, 512], BF16, tag="yt", bufs=2)
                        nc.vector.scalar_tensor_tensor(yt[:, :], pcT[g][:, :],
                                                       sin[:, g:g + 1],
                                                       y0T[g][:, :],
                                                       Alu.mult, Alu.add)
                        nc.tensor.matmul(py[:, :], r8S[:, g * 64:(g + 1) * 64],
                                         yt[:, :], start=(g == 0), stop=(g == 7))
                    yT = pB.tile([SC, 512], dt, name="yT", tag="yT")
                    nc.vector.tensor_copy(yT[:, :], py[:, :])

                    # ---- gate + out_proj + final residual (D-major) ----
                    yg = pB.tile([SC, 512], BF16, name="yg", tag="yg")
                    nc.vector.tensor_mul(yg[:, :], yT[:, :], gate_s[:, :])
                    for m in range(8):
                        p2 = psB.tile([128, 512], F32, tag="psB")
                        nc.tensor.matmul(p2[:, :], outW[:, m * 128:(m + 1) * 128],
                                         yg[:, :], start=True, stop=True)
                        yo = pB.tile([128, 512], dt, tag="yo", bufs=2)
                        nc.vector.tensor_add(yo[:, :], p2[:, :], x1d[m][:, :])
                        nc.sync.dma_start(yout[m * 128:(m + 1) * 128, :], yo[:, :])

    _split_multi_waits(nc)
    return nc


def kernel(x, qkv_w, o_w, norm1_w, norm2_w, in_w, out_w, A_log, Bp_w, Cp_w,
           dt_w, dt_b, gate_w):
    import ml_dtypes
    f = np.float32
    bf = ml_dtypes.bfloat16
    xf = np.ascontiguousarray(np.asarray(x, f).reshape(BT, D))
    xT_f = np.ascontiguousarray(xf.T)
    # pre-tiled x: [128, k, t] with row-block k of xT in column group k
    xTt = np.ascontiguousarray(
        xT_f.reshape(8, 128, BT).transpose(1, 0, 2)).astype(bf)
    # fold the rmsnorm elementwise weights into the consuming projections
    qkv_w1 = np.asarray(qkv_w, f) * np.asarray(norm1_w, f)[None, :]
    in_w2 = np.asarray(in_w, f) * np.asarray(norm2_w, f)[None, :]
    gate_w2 = np.asarray(gate_w, f) * np.asarray(norm2_w, f)[None, :]

    def tile128(wT):  # [D, W] -> [128, 8*W] with row-block k at column group k
        Dd, W = wT.shape
        return np.ascontiguousarray(
            wT.reshape(8, 128, W).transpose(1, 0, 2).reshape(128, 8 * W))

    oWt = tile128(np.asarray(o_w, f).T).astype(bf)
    inWt = tile128(in_w2.T).astype(bf)
    gateWt = tile128(gate_w2.T).astype(bf)
    out_wT = np.ascontiguousarray(np.asarray(out_w, f).T.astype(bf))
    dt_wT = np.ascontiguousarray(np.asarray(dt_w, f).T.astype(bf))
    BpT = np.ascontiguousarray(np.asarray(Bp_w, f).T.astype(bf))
    CpT = np.ascontiguousarray(np.asarray(Cp_w, f).T.astype(bf))
    dtbv = np.ascontiguousarray(np.asarray(dt_b, f).reshape(SC, 1))
    alogv = np.ascontiguousarray(np.asarray(A_log, f).reshape(1024).reshape(8, 128).T)
    ident = np.eye(128, dtype=f)
    tri_m = (np.arange(128)[None, :] >= np.arange(128)[:, None]).astype(f)
    cpkB = np.ascontiguousarray(
        np.concatenate([ident, tri_m, np.ones((128, 128), f)], axis=1).astype(bf))
    ejm = np.zeros((8, 1024), f)
    for j in range(8):
        ejm[j, j * 128:(j + 1) * 128] = 1.0
    jj = np.arange(1024)
    escm = (np.arange(SC)[:, None] == (jj[None, :] // 16)).astype(bf)
    estm = (np.arange(ST)[:, None] == (np.arange(128)[None, :] % 16)).astype(bf)
    r8m = np.zeros((128, 512), f)
    for g in range(8):
        for j in range(128):
            r8m[j, g * 64 + 8 * g + j // 16] = 1.0
    r8m = r8m.astype(bf)

    nc = _build()
    in_maps = []
    for c in range(NC):
        b, q = c // 4, c % 4
        h0 = 2 * c
        rows = np.concatenate([np.arange(h0 * 64, (h0 + 2) * 64),
                               D + np.arange(h0 * 64, (h0 + 2) * 64),
                               2 * D + np.arange(h0 * 64, (h0 + 2) * 64)])
        qkvWt = tile128(qkv_w1[rows, :].T).astype(bf)
        xmyT2 = np.ascontiguousarray(
            xT_f[:, c * TOK:(c + 1) * TOK].reshape(8, 128, TOK)
            .transpose(1, 0, 2).reshape(128, 8 * TOK))
        sel = np.zeros(NC, f)
        for j in range(q):
            sel[4 * b + j] = 1.0
        cselv = np.ascontiguousarray(np.tile(sel[None, :], (128, 1)))
        omcv = np.ascontiguousarray(
            np.repeat(1.0 - sel, 8)[None, :].repeat(128, axis=0).astype(f))
        in_maps.append({
            "xTt": xTt, "xmyT2": xmyT2, "qkvWt": qkvWt, "oWt": oWt,
            "inWt": inWt, "gateWt": gateWt, "out_wT": out_wT,
            "dt_wT": dt_wT, "BpT": BpT, "CpT": CpT,
            "dtb": dtbv, "alog": alogv, "cpkB": cpkB, "ej": ejm,
            "esc": escm, "est": estm, "r8": r8m,
            "csel": cselv, "omc": omcv,
            "epsb": np.full((128, 1), EPS, f),
        })
    import os
    trace = bool(int(os.environ.get("BASS_PROFILE", "0")))
    res = run_bass_kernel_spmd(nc, in_maps, core_ids=list(range(NC)),
                               trace=trace)
    if trace:
        print("HW exec time:", res.exec_time_ns, "ns")
        print("trace:", res.instructions_and_trace[1] if res.instructions_and_trace else None)
    out = np.concatenate([res.results[c]["yout"].T for c in range(NC)], axis=0)
    return out.reshape(B, T, D)


# revision 40
# speedup vs baseline: 1.1872x; 1.1856x over previous
import numpy as np
import concourse.bass as bass
import concourse.mybir as mybir
from concourse.bass_utils import run_bass_kernel_spmd
from concourse.tile import TileContext
from concourse.mybir import AluOpType as Alu, ActivationFunctionType as Act

B, T, D, H, hd, SC, ST = 2, 2048, 1024, 16, 64, 64, 16
BT = B * T          # 4096
NC = 8
TOK = BT // NC      # 512 tokens per core
EPS = 1.1920929e-07
F32 = mybir.dt.float32
BF16 = mybir.dt.bfloat16


def _split_multi_waits(nc, max_waits=1):
    # this walrus build accepts only one sync wait per ISA instruction
    n = 0
    for f in nc.m.functions:
        for bb in f.blocks:
            out = []
            for inst in bb.instructions:
                si = inst.sync_info
                if si is not None and si.on_wait and len(si.on_wait) > max_waits:
                    for w in si.on_wait[:-max_waits]:
                        out.append(mybir.InstNoOp(
                            name=f"{inst.name}_ws{n}", ins=[], outs=[],
                            engine=inst.engine,
                            sync_info=mybir.SyncInfo(on_wait=[w], on_update=[]),
                            bass_nofuse=True))
                        n += 1
                    inst.sync_info = mybir.SyncInfo(
                        on_wait=si.on_wait[-max_waits:], on_update=si.on_update)
                out.append(inst)
            bb.instructions = out
    return n


def _build():
    nc = bass.Bass()
    dt = mybir.dt.float32r if int(__import__("os").environ.get("BASS_F32R", "1")) else F32

    # pre-tiled inputs: [128, k*W+j] layouts so each stream is one DMA
    xTt = nc.dram_tensor("xTt", [128, 8, BT], BF16, kind="ExternalInput")
    xmyT2 = nc.dram_tensor("xmyT2", [128, 8 * TOK], dt, kind="ExternalInput")
    qkvWt = nc.dram_tensor("qkvWt", [128, 8 * 384], BF16, kind="ExternalInput")
    oWt = nc.dram_tensor("oWt", [128, 8 * D], BF16, kind="ExternalInput")
    inWt = nc.dram_tensor("inWt", [128, 8 * SC], BF16, kind="ExternalInput")
    gateWt = nc.dram_tensor("gateWt", [128, 8 * SC], BF16, kind="ExternalInput")
    out_wT = nc.dram_tensor("out_wT", [SC, D], BF16, kind="ExternalInput")
    dt_wT = nc.dram_tensor("dt_wT", [SC, SC], BF16, kind="ExternalInput")
    BpT = nc.dram_tensor("BpT", [SC, ST], BF16, kind="ExternalInput")
    CpT = nc.dram_tensor("CpT", [SC, ST], BF16, kind="ExternalInput")
    dtb = nc.dram_tensor("dtb", [SC, 1], F32, kind="ExternalInput")
    alog = nc.dram_tensor("alog", [128, 8], F32, kind="ExternalInput")
    cpkB = nc.dram_tensor("cpkB", [128, 384], BF16, kind="ExternalInput")
    ej = nc.dram_tensor("ej", [8, 1024], dt, kind="ExternalInput")
    esc = nc.dram_tensor("esc", [SC, 1024], BF16, kind="ExternalInput")
    est = nc.dram_tensor("est", [ST, 128], BF16, kind="ExternalInput")
    r8 = nc.dram_tensor("r8", [128, 512], BF16, kind="ExternalInput")
    csel = nc.dram_tensor("csel", [128, 8], F32, kind="ExternalInput")
    omc = nc.dram_tensor("omc", [128, 64], F32, kind="ExternalInput")
    epsb = nc.dram_tensor("epsb", [128, 1], F32, kind="ExternalInput")

    yout = nc.dram_tensor("yout", [D, TOK], dt, kind="ExternalOutput")

    with nc.allow_low_precision(reason="bf16 matmul paths; f32r is fp32 bytes"), \
         TileContext(nc) as tc:
        with tc.tile_pool(name="const", bufs=1) as cpool, \
             tc.tile_pool(name="wts", bufs=1) as wpool, \
             tc.tile_pool(name="ow", bufs=1) as owp, \
             tc.tile_pool(name="xmy", bufs=1) as xmp, \
             tc.tile_pool(name="work", bufs=2) as work, \
             tc.tile_pool(name="psA", bufs=2, space="PSUM") as psA, \
             tc.tile_pool(name="psB", bufs=3, space="PSUM") as psB, \
             tc.tile_pool(name="psC", bufs=2, space="PSUM") as psC, \
             tc.tile_pool(name="dram", bufs=1, space="DRAM") as dram:

            # ---- highest-priority DMAs first (single triggers each) ----
            xtmyA = xmp.tile([128, 8 * TOK], dt, name="xtmyA", tag="xtmyA")
            nc.sync.dma_start(xtmyA[:, :], xmyT2[:, :])

            # packed bf16 consts: ident | tri | ones
            cpkS = cpool.tile([128, 384], BF16, name="cpkS", tag="cpkS")
            nc.sync.dma_start(cpkS[:, :], cpkB[:, :])
            identS = cpkS[:, 0:128]
            triS = cpkS[:, 128:256]
            onesS = cpkS[:, 256:384]

            def csbuf(shape, src, name, d=None):
                t = cpool.tile(shape, d or dt, name=name, tag=name)
                nc.sync.dma_start(t[:, :], src)
                return t

            ejS = csbuf([8, 1024], ej[:, :], "ejS")
            escS = csbuf([SC, 1024], esc[:, :], "escS", BF16)
            estS = csbuf([ST, 128], est[:, :], "estS", BF16)
            r8S = csbuf([128, 512], r8[:, :], "r8S", BF16)
            cselS = csbuf([128, 8], csel[:, :], "cselS", F32)
            omcS = csbuf([128, 64], omc[:, :], "omcS", F32)
            dtbS = csbuf([SC, 1], dtb[:, :], "dtbS", F32)
            alogS = csbuf([128, 8], alog[:, :], "alogS", F32)
            epsS = csbuf([128, 1], epsb[:, :], "epsS", F32)

            zerosF = cpool.tile([128, 512], F32, name="zerosF", tag="zerosF")
            nc.vector.memset(zerosF[:, :], 0.0)
            negA = cpool.tile([128, 8], F32, name="negA", tag="negA")
            nc.scalar.activation(negA[:, :], alogS[:, :], Act.Exp)
            nc.vector.tensor_scalar_mul(negA[:, :], negA[:, :], -1.0)

            # persistent post-phase weight tiles (DMAs issued later)
            inW = wpool.tile([128, 8 * SC], BF16, name="inW", tag="inW")
            gateW = wpool.tile([128, 8 * SC], BF16, name="gateW", tag="gateW")
            outW = wpool.tile([SC, D], BF16, name="outW", tag="outW")
            dtW = wpool.tile([SC, SC], BF16, name="dtW", tag="dtW")
            BpS = wpool.tile([SC, ST], BF16, name="BpS", tag="BpS")
            CpS = wpool.tile([SC, ST], BF16, name="CpS", tag="CpS")
            # o_proj weights (bf16, one trigger; sync queue so the gpsimd
            # queue reaches the scale-AllGather trigger promptly)
            oW = owp.tile([128, 8 * D], BF16, name="oW", tag="oW")
            nc.sync.dma_start(oW[:, :], oWt[:, :])

            # dram collective buffers; attention output travels as two
            # half-height AllToAlls so the first overlaps the second half
            # of attention compute
            cin1 = dram.tile([1, 512], BF16, name="cin1T", tag="cin1T")
            cout1 = dram.tile([NC, 512], BF16, name="cout1T", tag="cout1T")
            cinH = [dram.tile([NC, 64, TOK], BF16, name=f"cinH{h}", tag=f"cinH{h}")
                    for h in range(2)]
            coutH = [dram.tile([NC, 64, TOK], BF16, name=f"coutH{h}",
                               tag=f"coutH{h}") for h in range(2)]
            cin2 = dram.tile([128, 16], F32, name="cin2T", tag="cin2T")
            cout2 = dram.tile([NC * 128, 16], F32, name="cout2T", tag="cout2T")

            with tc.tile_pool(name="attn", bufs=1) as apool:
                qkvW = apool.tile([128, 8 * 384], BF16, name="qkvW", tag="qkvW")
                nc.sync.dma_start(qkvW[:, :], qkvWt[:, :])
                Qf = [apool.tile([128, T], BF16, name=f"Qf{b}", tag=f"Qf{b}")
                      for b in range(B)]
                # K stored zero-padded per head: Kz[b][hh] keeps head hh's 64
                # rows in place and zeros the other head's rows, so score
                # matmuls contract over K=128 (full-rate on the PE)
                Kz = [[apool.tile([128, T], BF16, name=f"Kz{b}_{hh}",
                                  tag=f"Kz{b}_{hh}") for hh in range(2)]
                      for b in range(B)]
                Vraw = [[apool.tile([128, 512], BF16, name=f"Vr{b}_{blk}",
                                    tag=f"Vr{b}_{blk}") for blk in range(4)]
                        for b in range(B)]
                # V^T tiles padded to 128 output columns (full-rate M=128):
                # cols 0-63 = V^T, col 64 = ones (denominator), 65-127 = zeros
                Vp = [[[apool.tile([128, 128], BF16, name=f"Vp{b}_{hh}_{kt}",
                                   tag=f"Vp{b}_{hh}_{kt}") for kt in range(16)]
                       for hh in range(2)] for b in range(B)]
                for b in range(B):
                    for hh in range(2):
                        z0, z1 = (64, 128) if hh == 0 else (0, 64)
                        for cc in range(4):
                            nc.vector.tensor_copy(
                                Kz[b][hh][z0:z1, cc * 512:(cc + 1) * 512],
                                zerosF[z0:z1, :])
                        for kt in range(16):
                            nc.vector.tensor_copy(Vp[b][hh][kt][:, 65:128],
                                                  zerosF[:, 0:63])

                # rmsnorm1 sums first so the scale AllGather fires early and
                # hides under the qkv matmuls
                ssp = psA.tile([1, 512], F32, tag="psA")
                for k in range(8):
                    sq = work.tile([128, 512], BF16, tag="sq", bufs=2)
                    nc.scalar.activation(sq[:, :],
                                         xtmyA[:, k * 512:(k + 1) * 512],
                                         Act.Square)
                    nc.tensor.matmul(ssp[:, :], onesS[:, 0:1], sq[:, :],
                                     start=(k == 0), stop=(k == 7))
                lnm = work.tile([1, 512], F32, tag="lnm", bufs=1)
                nc.scalar.activation(lnm[:, :], ssp[:, :], Act.Ln,
                                     scale=1.0 / D, bias=epsS[0:1, :])
                srow = work.tile([1, 512], BF16, tag="srow", bufs=1)
                nc.scalar.activation(srow[:, :], lnm[:, :], Act.Exp, scale=-0.5)
                nc.gpsimd.dma_start(cin1[:, :], srow[:, :])
                nc.gpsimd.collective_compute(
                    "AllGather", Alu.bypass, [list(range(NC))],
                    ins=[cin1.opt()], outs=[cout1.opt()])
                sAG = wpool.tile([1, NC * 512], BF16, name="sAG", tag="sAG")
                nc.gpsimd.dma_start(sAG[0:1, :], cout1[:, :])

                # ---- qkv for BOTH batches before attention ----
                for b in range(B):
                    for blk in range(4):
                        j = 4 * b + blk
                        xta = apool.tile([128, 8 * 512], BF16, name="xta",
                                         tag="xta", bufs=3)
                        nc.sync.dma_start(xta[:, :],
                                          xTt[:, :, j * 512:(j + 1) * 512])
                        for m in range(3):
                            om = psB.tile([128, 512], F32, tag="psB")
                            for k in range(8):
                                nc.tensor.matmul(
                                    om[:, :],
                                    qkvW[:, k * 384 + m * 128:k * 384 + (m + 1) * 128],
                                    xta[:, k * 512:(k + 1) * 512],
                                    start=(k == 0), stop=(k == 7))
                            if m == 0:
                                nc.scalar.copy(Qf[b][:, blk * 512:(blk + 1) * 512],
                                               om[:, :])
                            elif m == 1:
                                cs_ = slice(blk * 512, (blk + 1) * 512)
                                nc.scalar.copy(Kz[b][0][0:64, cs_], om[0:64, :])
                                nc.scalar.copy(Kz[b][1][64:128, cs_],
                                               om[64:128, :])
                            else:
                                nc.scalar.copy(Vraw[b][blk][:, :], om[:, :])

                # ---- apply the scales; build V^T tiles ----
                for b in range(B):
                    for blk in range(4):
                        j = 4 * b + blk
                        rsp = psA.tile([128, 512], F32, tag="psA")
                        nc.tensor.matmul(rsp[:, :], onesS[0:1, 0:128],
                                         sAG[0:1, j * 512:(j + 1) * 512],
                                         start=True, stop=True)
                        rsbS = apool.tile([128, 512], dt, tag="rsb", bufs=2)
                        nc.scalar.copy(rsbS[:, :], rsp[:, :])
                        cs = slice(blk * 512, (blk + 1) * 512)
                        nc.vector.tensor_mul(Qf[b][:, cs], Qf[b][:, cs], rsbS[:, :])
                        nc.vector.tensor_mul(Kz[b][0][0:64, cs], Kz[b][0][0:64, cs],
                                             rsbS[0:64, :])
                        nc.vector.tensor_mul(Kz[b][1][64:128, cs],
                                             Kz[b][1][64:128, cs],
                                             rsbS[64:128, :])
                        vfb = apool.tile([128, 512], BF16, tag="vfb", bufs=2)
                        nc.vector.tensor_mul(vfb[:, :], Vraw[b][blk][:, :],
                                             rsbS[:, :])
                        for sub in range(4):
                            kt = blk * 4 + sub
                            for hh in range(2):
                                vtp = psC.tile([128, 64], BF16, tag="psC")
                                nc.tensor.transpose(
                                    vtp[:, :],
                                    vfb[64 * hh:64 * hh + 64,
                                        sub * 128:(sub + 1) * 128],
                                    identS[64 * hh:64 * hh + 64,
                                           64 * hh:64 * hh + 64])
                                nc.vector.tensor_copy(Vp[b][hh][kt][:, 0:64],
                                                      vtp[:, :])
                                nc.vector.tensor_copy(Vp[b][hh][kt][:, 64:65],
                                                      onesS[:, 0:1])

                # ---- attention (hh-major: the first head-half's A2A fires
                # while the second half computes) ----
                for hh in range(2):
                    for b in range(B):
                        r0 = 64 * hh
                        for qb in range(4):
                            q0 = qb * 512
                            ops = psC.tile([128, 512], F32, tag="psC")
                            nkt = 4 * qb + 4

                            def score_mm(kt):
                                # causal: queries before the key block never
                                # read these scores, so skip those columns
                                c0 = 128 * max(0, kt - 4 * qb)
                                sp = psB.tile([128, 512], F32, tag="psB")
                                nc.tensor.matmul(
                                    sp[:, c0:512],
                                    Kz[b][hh][:, kt * 128:(kt + 1) * 128],
                                    Qf[b][:, q0 + c0:q0 + 512],
                                    start=True, stop=True)
                                return sp

                            # 2-deep score prefetch: keeps the exp latency off
                            # the PE critical path (psB ring is 3 banks)
                            spq = [score_mm(kt) for kt in range(min(2, nkt))]
                            for kt in range(nkt):
                                sp = spq.pop(0)
                                if kt + 2 < nkt:
                                    spq.append(score_mm(kt + 2))
                                e = apool.tile([128, 512], BF16, tag="expst",
                                               bufs=4)
                                d = kt - 4 * qb
                                if d < 0:
                                    nc.scalar.activation(e[:, :], sp[:, :],
                                                         Act.Exp, scale=0.125)
                                    nc.tensor.matmul(ops[:, :],
                                                     Vp[b][hh][kt][:, :],
                                                     e[:, :], start=(kt == 0),
                                                     stop=False)
                                else:
                                    nc.scalar.activation(e[:, 128 * d:512],
                                                         sp[:, 128 * d:512],
                                                         Act.Exp, scale=0.125)
                                    nc.vector.tensor_mul(
                                        e[:, 128 * d:128 * (d + 1)],
                                        e[:, 128 * d:128 * (d + 1)], triS[:, :])
                                    nc.tensor.matmul(ops[:, 128 * d:512],
                                                     Vp[b][hh][kt][:, :],
                                                     e[:, 128 * d:512],
                                                     start=(kt == 0),
                                                     stop=(kt == nkt - 1),
                                                     skip_group_check=True)
                            # 1/denominator = exp(-ln(den)) on Act engine
                            lnd = apool.tile([1, 512], F32, tag="lnd", bufs=2)
                            nc.scalar.activation(lnd[:, :], ops[64:65, :], Act.Ln)
                            rle = apool.tile([1, 512], BF16, tag="rle", bufs=2)
                            nc.scalar.activation(rle[:, :], lnd[:, :], Act.Exp,
                                                 scale=-1.0)
                            rb = psA.tile([64, 512], F32, tag="psA")
                            nc.tensor.matmul(rb[:, :], onesS[0:1, 0:64],
                                             rle[:, :], start=True, stop=True)
                            rbc = apool.tile([64, 512], dt, tag="rbc", bufs=2)
                            nc.vector.tensor_copy(rbc[:, :], rb[:, :])
                            rbs = apool.tile([64, 512], BF16, tag="rbs", bufs=2)
                            nc.vector.tensor_mul(rbs[:, :], ops[0:64, :],
                                                 rbc[:, :])
                            nc.sync.dma_start(cinH[hh][4 * b + qb, :, :],
                                              rbs[:, :])
                    if hh == 0:
                        nc.gpsimd.collective_compute(
                            "AllToAll", Alu.bypass, [list(range(NC))],
                            ins=[cinH[0].opt()], outs=[coutH[0].opt()])

            # post-phase weights: issue now, overlapping the A2A
            nc.sync.dma_start(inW[:, :], inWt[:, :])
            nc.sync.dma_start(gateW[:, :], gateWt[:, :])
            nc.sync.dma_start(outW[:, :], out_wT[:, :])
            nc.sync.dma_start(dtW[:, :], dt_wT[:, :])
            nc.sync.dma_start(BpS[:, :], BpT[:, :])
            nc.sync.dma_start(CpS[:, :], CpT[:, :])

            nc.gpsimd.collective_compute(
                "AllToAll", Alu.bypass, [list(range(NC))],
                ins=[cinH[1].opt()], outs=[coutH[1].opt()])

            with tc.tile_pool(name="xpool", bufs=1) as xpool:
                x1d = [xpool.tile([128, 512], dt, name=f"x1d{m}", tag=f"x1d{m}")
                       for m in range(8)]
                h2T = [xpool.tile([128, 512], BF16, name=f"h2T{k}", tag=f"h2T{k}")
                       for k in range(8)]

                with tc.tile_pool(name="postA", bufs=1) as pA:
                    # attention rows for my tokens, loaded once (bf16)
                    otg = [pA.tile([128, 512], BF16, name=f"otg{k}", tag=f"otg{k}")
                           for k in range(8)]
                    for k in range(8):
                        nc.sync.dma_start(otg[k][0:64, :], coutH[0][k, :, :])
                        nc.sync.dma_start(otg[k][64:128, :], coutH[1][k, :, :])
                    # o_proj + residual (D-major)
                    for m in range(8):
                        pr = psB.tile([128, 512], F32, tag="psB")
                        for k in range(8):
                            nc.tensor.matmul(
                                pr[:, :],
                                oW[:, k * D + m * 128:k * D + (m + 1) * 128],
                                otg[k][:, :], start=(k == 0), stop=(k == 7))
                        nc.vector.tensor_add(x1d[m][:, :], pr[:, :],
                                             xtmyA[:, m * 512:(m + 1) * 512])

                # ---- rmsnorm2 (D-major) ----
                ssp2 = psA.tile([1, 512], F32, tag="psA")
                for k in range(8):
                    sq2 = work.tile([128, 512], BF16, tag="sq", bufs=2)
                    nc.scalar.activation(sq2[:, :], x1d[k][:, :], Act.Square)
                    nc.tensor.matmul(ssp2[:, :], onesS[:, 0:1], sq2[:, :],
                                     start=(k == 0), stop=(k == 7))
                ln2 = work.tile([1, 512], F32, tag="lnm", bufs=1)
                nc.scalar.activation(ln2[:, :], ssp2[:, :], Act.Ln,
                                     scale=1.0 / D, bias=epsS[0:1, :])
                s2row = work.tile([1, 512], BF16, tag="srow2", bufs=1)
                nc.scalar.activation(s2row[:, :], ln2[:, :], Act.Exp, scale=-0.5)
                rs2p = psA.tile([128, 512], F32, tag="psA")
                nc.tensor.matmul(rs2p[:, :], onesS[0:1, 0:128], s2row[:, :],
                                 start=True, stop=True)
                rsb2S = work.tile([128, 512], dt, tag="rsb2", bufs=1)
                nc.scalar.copy(rsb2S[:, :], rs2p[:, :])
                for k in range(8):
                    nc.vector.tensor_mul(h2T[k][:, :], x1d[k][:, :], rsb2S[:, :])

                with tc.tile_pool(name="postB", bufs=1) as pB:
                    # ---- scan projections (bf16 matmuls) ----
                    pz = psB.tile([SC, 512], F32, tag="psB")
                    for k in range(8):
                        nc.tensor.matmul(pz[:, :], inW[:, k * SC:(k + 1) * SC],
                                         h2T[k][:, :], start=(k == 0), stop=(k == 7))
                    z_s = pB.tile([SC, 512], BF16, name="z_s", tag="z_s")
                    nc.vector.tensor_copy(z_s[:, :], pz[:, :])
                    pdt = psB.tile([SC, 512], F32, tag="psB")
                    nc.tensor.matmul(pdt[:, :], dtW[:, :], z_s[:, :],
                                     start=True, stop=True)
                    dt_s = pB.tile([SC, 512], BF16, name="dt_s", tag="dt_s")
                    nc.scalar.activation(dt_s[:, :], pdt[:, :], Act.Exp,
                                         bias=dtbS[:, :])
                    nc.scalar.activation(dt_s[:, :], dt_s[:, :], Act.Ln, bias=1.0)
                    dtz_s = pB.tile([SC, 512], BF16, name="dtz_s", tag="dtz_s")
                    nc.vector.tensor_mul(dtz_s[:, :], dt_s[:, :], z_s[:, :])
                    pbi = psB.tile([ST, 512], F32, tag="psB")
                    nc.tensor.matmul(pbi[:, :], BpS[:, :], z_s[:, :],
                                     start=True, stop=True)
                    bi_s = pB.tile([ST, 512], BF16, name="bi_s", tag="bi_s")
                    nc.vector.tensor_copy(bi_s[:, :], pbi[:, :])
                    pci = psB.tile([ST, 512], F32, tag="psB")
                    nc.tensor.matmul(pci[:, :], CpS[:, :], z_s[:, :],
                                     start=True, stop=True)
                    ci_s = pB.tile([ST, 512], BF16, name="ci_s", tag="ci_s")
                    nc.vector.tensor_copy(ci_s[:, :], pci[:, :])

                    # ---- local scans (zero-init trajectories + cumprods) ----
                    ppT = [pB.tile([128, 512], BF16, name=f"ppT{g}", tag=f"ppT{g}")
                           for g in range(8)]
                    sc0T = [pB.tile([128, 512], dt, name=f"sc0T{g}",
                                    tag=f"sc0T{g}") for g in range(8)]
                    stg2 = pB.tile([128, 16], F32, name="stg2", tag="stg2")
                    for g in range(8):
                        pde = psB.tile([128, 512], F32, tag="psB")
                        nc.tensor.matmul(pde[:, :], escS[:, g * 128:(g + 1) * 128],
                                         dt_s[:, :], start=True, stop=True)
                        abar = pB.tile([128, 512], dt, tag="abar", bufs=3)
                        nc.scalar.activation(abar[:, :], pde[:, :], Act.Identity,
                                             scale=negA[:, g:g + 1], bias=1.0)
                        pdz = psB.tile([128, 512], F32, tag="psB")
                        nc.tensor.matmul(pdz[:, :], escS[:, g * 128:(g + 1) * 128],
                                         dtz_s[:, :], start=True, stop=True)
                        pbe = psC.tile([128, 512], F32, tag="psC")
                        nc.tensor.matmul(pbe[:, :], estS[:, :], bi_s[:, :],
                                         start=True, stop=True)
                        bes = pB.tile([128, 512], dt, tag="bes", bufs=3)
                        nc.scalar.copy(bes[:, :], pbe[:, :])
                        bin_ = pB.tile([128, 512], dt, tag="bin_", bufs=3)
                        nc.vector.tensor_mul(bin_[:, :], pdz[:, :], bes[:, :])
                        nc.vector.tensor_tensor_scan(sc0T[g][:, :], abar[:, :],
                                                     bin_[:, :], 0.0,
                                                     Alu.mult, Alu.add)
                        nc.vector.tensor_tensor_scan(ppT[g][:, :], abar[:, :],
                                                     abar[:, :], 1.0,
                                                     Alu.mult, Alu.bypass)
                        nc.vector.tensor_copy(stg2[:, g:g + 1],
                                              ppT[g][:, 511:512])
                        nc.vector.tensor_copy(stg2[:, 8 + g:8 + g + 1],
                                              sc0T[g][:, 511:512])

                    nc.gpsimd.dma_start(cin2[:, :], stg2[:, :])
                    nc.gpsimd.collective_compute(
                        "AllGather", Alu.bypass, [list(range(NC))],
                        ins=[cin2.opt()], outs=[cout2.opt()])

                    # overlap the AllGather with work that doesn't need it:
                    # C embedding and the gate (single act-table switch to Silu)
                    ces = pB.tile([128, 512], BF16, name="ces", tag="ces")
                    pce = psC.tile([128, 512], F32, tag="psC")
                    nc.tensor.matmul(pce[:, :], estS[:, :], ci_s[:, :],
                                     start=True, stop=True)
                    nc.vector.tensor_copy(ces[:, :], pce[:, :])
                    pg = psB.tile([SC, 512], F32, tag="psB")
                    for k in range(8):
                        nc.tensor.matmul(pg[:, :], gateW[:, k * SC:(k + 1) * SC],
                                         h2T[k][:, :], start=(k == 0), stop=(k == 7))
                    gate_s = pB.tile([SC, 512], BF16, name="gate_s", tag="gate_s")
                    nc.scalar.activation(gate_s[:, :], pg[:, :], Act.Silu)
                    # y = sc0*ces + sin*(pp*ces): precompute both products
                    y0T = [pB.tile([128, 512], BF16, name=f"y0T{g}", tag=f"y0T{g}")
                           for g in range(8)]
                    pcT = [pB.tile([128, 512], BF16, name=f"pcT{g}", tag=f"pcT{g}")
                           for g in range(8)]
                    for g in range(8):
                        nc.vector.tensor_mul(y0T[g][:, :], sc0T[g][:, :],
                                             ces[:, :])
                        nc.vector.tensor_mul(pcT[g][:, :], ppT[g][:, :],
                                             ces[:, :])

                    # ---- stitch initial states from preceding cores ----
                    pjs = []
                    for jj in range(NC):
                        pj = work.tile([128, 16], F32, tag=f"pj{jj}", bufs=1)
                        nc.gpsimd.dma_start(pj[:, :],
                                            cout2[jj * 128:(jj + 1) * 128, :])
                        pjs.append(pj)
                    sin = pB.tile([128, 8], F32, name="sin", tag="sin")
                    nc.vector.memset(sin[:, :], 0.0)
                    for jj in range(NC):
                        pe_ = work.tile([128, 8], F32, tag="pe_")
                        nc.vector.scalar_tensor_tensor(pe_[:, :], pjs[jj][:, 0:8],
                                                       cselS[:, jj:jj + 1],
                                                       omcS[:, 8 * jj:8 * jj + 8],
                                                       Alu.mult, Alu.add)
                        se_ = work.tile([128, 8], F32, tag="se_")
                        nc.vector.tensor_scalar_mul(se_[:, :], pjs[jj][:, 8:16],
                                                    cselS[:, jj:jj + 1])
                        nc.vector.tensor_mul(sin[:, :], sin[:, :], pe_[:, :])
                        nc.vector.tensor_add(sin[:, :], sin[:, :], se_[:, :])

                    # ---- finalize: y_g = y0 + sin_g * pc ----
                    py = psA.tile([SC, 512], F32, tag="psA")
                    for g in range(8):
                        yt = pB.tile([128, 512], BF16, tag="yt", bufs=2)
                        nc.vector.scalar_tensor_tensor(yt[:, :], pcT[g][:, :],
                                                       sin[:, g:g + 1],
                                                       y0T[g][:, :],
                                                       Alu.mult, Alu.add)
                        nc.tensor.matmul(py[:, :], r8S[:, g * 64:(g + 1) * 64],
                                         yt[:, :], start=(g == 0), stop=(g == 7))
                    yT = pB.tile([SC, 512], dt, name="yT", tag="yT")
                    nc.vector.tensor_copy(yT[:, :], py[:, :])

                    # ---- gate + out_proj + final residual (D-major) ----
                    yg = pB.tile([SC, 512], BF16, name="yg", tag="yg")
                    nc.vector.tensor_mul(yg[:, :], yT[:, :], gate_s[:, :])
                    for m in range(8):
                        p2 = psB.tile([128, 512], F32, tag="psB")
                        nc.tensor.matmul(p2[:, :], outW[:, m * 128:(m + 1) * 128],
                                         yg[:, :], start=True, stop=True)
                        yo = pB.tile([128, 512], dt, tag="yo", bufs=2)
                        nc.vector.tensor_add(yo[:, :], p2[:, :], x1d[m][:, :])
                        nc.sync.dma_start(yout[m * 128:(m + 1) * 128, :], yo[:, :])

    _split_multi_waits(nc)
    return nc


def kernel(x, qkv_w, o_w, norm1_w, norm2_w, in_w, out_w, A_log, Bp_w, Cp_w,
           dt_w, dt_b, gate_w):
    import ml_dtypes
    f = np.float32
    bf = ml_dtypes.bfloat16
    xf = np.ascontiguousarray(np.asarray(x, f).reshape(BT, D))
    xT_f = np.ascontiguousarray(xf.T)
    # pre-tiled x: [128, k, t] with row-block k of xT in column group k
    xTt = np.ascontiguousarray(
        xT_f.reshape(8, 128, BT).transpose(1, 0, 2)).astype(bf)
    # fold the rmsnorm elementwise weights into the consuming projections
    qkv_w1 = np.asarray(qkv_w, f) * np.asarray(norm1_w, f)[None, :]
    in_w2 = np.asarray(in_w, f) * np.asarray(norm2_w, f)[None, :]
    gate_w2 = np.asarray(gate_w, f) * np.asarray(norm2_w, f)[None, :]

    def tile128(wT):  # [D, W] -> [128, 8*W] with row-block k at column group k
        Dd, W = wT.shape
        return np.ascontiguousarray(
            wT.reshape(8, 128, W).transpose(1, 0, 2).reshape(128, 8 * W))

    oWt = tile128(np.asarray(o_w, f).T).astype(bf)
    inWt = tile128(in_w2.T).astype(bf)
    gateWt = tile128(gate_w2.T).astype(bf)
    out_wT = np.ascontiguousarray(np.asarray(out_w, f).T.astype(bf))
    dt_wT = np.ascontiguousarray(np.asarray(dt_w, f).T.astype(bf))
    BpT = np.ascontiguousarray(np.asarray(Bp_w, f).T.astype(bf))
    CpT = np.ascontiguousarray(np.asarray(Cp_w, f).T.astype(bf))
    dtbv = np.ascontiguousarray(np.asarray(dt_b, f).reshape(SC, 1))
    alogv = np.ascontiguousarray(np.asarray(A_log, f).reshape(1024).reshape(8, 128).T)
    ident = np.eye(128, dtype=f)
    tri_m = (np.arange(128)[None, :] >= np.arange(128)[:, None]).astype(f)
    cpkB = np.ascontiguousarray(
        np.concatenate([ident, tri_m, np.ones((128, 128), f)], axis=1).astype(bf))
    ejm = np.zeros((8, 1024), f)
    for j in range(8):
        ejm[j, j * 128:(j + 1) * 128] = 1.0
    jj = np.arange(1024)
    escm = (np.arange(SC)[:, None] == (jj[None, :] // 16)).astype(bf)
    estm = (np.arange(ST)[:, None] == (np.arange(128)[None, :] % 16)).astype(bf)
    r8m = np.zeros((128, 512), f)
    for g in range(8):
        for j in range(128):
            r8m[j, g * 64 + 8 * g + j // 16] = 1.0
    r8m = r8m.astype(bf)

    nc = _build()
    in_maps = []
    for c in range(NC):
        b, q = c // 4, c % 4
        h0 = 2 * c
        rows = np.concatenate([np.arange(h0 * 64, (h0 + 2) * 64),
                               D + np.arange(h0 * 64, (h0 + 2) * 64),
                               2 * D + np.arange(h0 * 64, (h0 + 2) * 64)])
        qkvWt = tile128(qkv_w1[rows, :].T).astype(bf)
        xmyT2 = np.ascontiguousarray(
            xT_f[:, c * TOK:(c + 1) * TOK].reshape(8, 128, TOK)
            .transpose(1, 0, 2).reshape(128, 8 * TOK))
        sel = np.zeros(NC, f)
        for j in range(q):
            sel[4 * b + j] = 1.0
        cselv = np.ascontiguousarray(np.tile(sel[None, :], (128, 1)))
        omcv = np.ascontiguousarray(
            np.repeat(1.0 - sel, 8)[None, :].repeat(128, axis=0).astype(f))
        in_maps.append({
            "xTt": xTt, "xmyT2": xmyT2, "qkvWt": qkvWt, "oWt": oWt,
            "inWt": inWt, "gateWt": gateWt, "out_wT": out_wT,
            "dt_wT": dt_wT, "BpT": BpT, "CpT": CpT,
            "dtb": dtbv, "alog": alogv, "cpkB": cpkB, "ej": ejm,
            "esc": escm, "est": estm, "r8": r8m,
            "csel": cselv, "omc": omcv,
            "epsb": np.full((128, 1), EPS, f),
        })
    import os
    trace = bool(int(os.environ.get("BASS_PROFILE", "0")))
    res = run_bass_kernel_spmd(nc, in_maps, core_ids=list(range(NC)),
                               trace=trace)
    if trace:
        print("HW exec time:", res.exec_time_ns, "ns")
        print("trace:", res.instructions_and_trace[1] if res.instructions_and_trace else None)
    out = np.concatenate([res.results[c]["yout"].T for c in range(NC)], axis=0)
    return out.reshape(B, T, D)
